# revision 2
# baseline (speedup 1.0000x reference)
"""Trainium2 Bass kernel for Local_Scale_Attention (v2).

Problem (hardcoded shapes):
  x:          (2048, 84, 256) f32
  W_qkv:      (256, 768) f32,  b_qkv: (768,) f32
  W_proj:     (256, 256) f32,  b_proj: (256,) f32
  bias_table: (207, 8) f32,    rel_index: (84, 84) i32
  out:        (2048, 84, 256) f32

Sharding: data-parallel over batch across 8 cores (256 batches/core).

v2 changes vs v1:
  - q,k projections in fp8e4m3 with DoubleRow perf mode (K=256 in one
    pass, 0.5 cyc/col): host sends xT8 (e4m3) and W_qk * 32 (e4m3);
    logits come out scaled by 1024, folded into the exp scale.
    v and proj matmuls stay bf16 (fp8 there fails the 2e-2 error gate).
  - score fills split into half-fills (3 batches x 4 heads = 2 PSUM
    banks) double-buffered, so exp granularity halves and the PE never
    waits on a psum drain.
  - 2-group software pipeline skew: group i's qkv/v/scores interleave
    with group i-2's AV/proj, giving dependency chains ~2 group-times
    of slack so the PE stays continuously busy (p-state ramp!).
  - explicit engine assignment: ACT = exp + qk evac (bias add),
    DVE = v/y evac + reciprocal + O/D divide, GPSIMD = E0*exp(bias)
    (gpsimd cannot touch PSUM; the E-mult is the big SBUF-only op).
"""

import math
import os

import numpy as np
import ml_dtypes

USE_FP8 = os.environ.get("K2_FP8", "1") == "1"
EMULT_GPS = os.environ.get("K2_EMULT_GPS", "1") == "1"
QKEVAC_ACT = os.environ.get("K2_QKEVAC_ACT", "1") == "1"

import concourse.bass as bass
import concourse.bacc as bacc
import concourse.mybir as mybir
import concourse.tile as tile
from concourse.bass_utils import run_bass_kernel_spmd

F32 = mybir.dt.float32
BF16 = mybir.dt.bfloat16
F8 = mybir.dt.float8e4

N_CORES = 8
B_TOTAL = 2048
B_SHARD = B_TOTAL // N_CORES  # 256
N_TOK = 84
DIM = 256
H = 8
HD = 32
SCALE = float(N_TOK) ** -0.5

GROUP = 6           # batches per outer group (N = 6*84 = 504 <= 512)
FILL = 3            # batches per S-psum fill (12 S-matrices in 2 banks/half)

WQK_SCALE = 32.0 if USE_FP8 else 1.0  # fp8 pre-scale on W_q/W_k
LOGIT_SCALE = SCALE / (WQK_SCALE * WQK_SCALE)

DR = mybir.MatmulPerfMode.DoubleRow


def build_nc(n_batches=B_SHARD):
    T_ALL = n_batches * N_TOK
    nc = bacc.Bacc("TRN2", target_bir_lowering=False, debug=False)

    QK_DT = F8 if USE_FP8 else BF16
    xT8 = nc.declare_dram_parameter("xT8", [DIM, T_ALL], QK_DT, isOutput=False)
    xT = nc.declare_dram_parameter("xT", [DIM, T_ALL], BF16, isOutput=False)
    wqk8 = nc.declare_dram_parameter("wqk8", [DIM, 2 * DIM], QK_DT, isOutput=False)
    wv = nc.declare_dram_parameter("wv", [DIM, DIM], BF16, isOutput=False)
    bqk = nc.declare_dram_parameter("bqk", [2 * DIM], F32, isOutput=False)
    wproj = nc.declare_dram_parameter("wproj", [DIM, DIM], BF16, isOutput=False)
    # exp(bias) pre-aligned to the S-fill layout (bank = h%4, slot parity
    # = h//4): [84(k), 4(bank), 504(6 slots x 84 q)]
    ebias = nc.declare_dram_parameter("ebias", [N_TOK, 4, 504], BF16,
                                      isOutput=False)
    yT = nc.declare_dram_parameter("yT", [DIM, T_ALL], BF16, isOutput=True)

    groups = []
    b0 = 0
    while b0 < n_batches:
        groups.append((b0, min(GROUP, n_batches - b0)))
        b0 += GROUP

    with tile.TileContext(nc) as tc:
        with (
            tc.tile_pool(name="const", bufs=1) as const,
            tc.tile_pool(name="xin8", bufs=4) as xin8,
            tc.tile_pool(name="xinb", bufs=4) as xinb,
            tc.tile_pool(name="qk", bufs=3) as qk_pool,
            tc.tile_pool(name="vsb", bufs=4) as v_pool,
            tc.tile_pool(name="e0sb", bufs=8) as e0_pool,
            tc.tile_pool(name="esb", bufs=20) as e_pool,
            tc.tile_pool(name="otsb", bufs=3) as ot_pool,
            tc.tile_pool(name="ysb", bufs=3) as y_pool,
            tc.tile_pool(name="mm_ps", bufs=8, space="PSUM") as mm_ps,
        ):
            # ---- static tiles ----
            wqk8_sb = const.tile([128, 2, 2 * DIM], QK_DT)
            nc.sync.dma_start(wqk8_sb, wqk8.rearrange("(kc p) m -> p kc m", p=128))
            wv_sb = const.tile([128, 2, DIM], BF16)
            nc.sync.dma_start(wv_sb, wv.rearrange("(kc p) m -> p kc m", p=128))
            wproj_sb = const.tile([128, 2, DIM], BF16)
            nc.sync.dma_start(wproj_sb, wproj.rearrange("(kc p) m -> p kc m", p=128))
            bqk_sb = const.tile([128, 4], F32)
            nc.sync.dma_start(bqk_sb, bqk.rearrange("(m p) -> p m", p=128))
            eb_sb = const.tile([N_TOK, 4, 504], BF16)
            nc.sync.dma_start(eb_sb, ebias[:])
            ones_sb = const.tile([N_TOK, HD], BF16)
            nc.vector.memset(ones_sb, 1.0)

            # ---------- pipelined stage helpers ----------
            def emit_qkv_m(xt8, g, ms):
                """q/k projection bands. m=0,1: q feats 0:128/128:256;
                m=2,3: same for k.  fp8 DoubleRow does K=256 in one pass.
                Evac: even m on ACT, odd m on DVE (scores need all four)."""
                TG = g * N_TOK
                for m in ms:
                    ps = mm_ps.tile([128, 512], F32, tag="mmps")
                    if USE_FP8:
                        nc.tensor.matmul(
                            ps[:, :TG],
                            wqk8_sb[:, :, m * 128:(m + 1) * 128],
                            xt8[:, :, :TG],
                            start=True, stop=True, perf_mode=DR,
                        )
                    else:
                        for kc in range(2):
                            nc.tensor.matmul(
                                ps[:, :TG],
                                wqk8_sb[:, kc, m * 128:(m + 1) * 128],
                                xt8[:, kc, :TG],
                                start=(kc == 0), stop=(kc == 1),
                            )
                    if m % 2 == 0:
                        nc.scalar.add(
                            qk_sb_cur[0][:, m, :TG], ps[:, :TG],
                            bqk_sb[:, m:m + 1]
                        )
                    else:
                        nc.vector.tensor_scalar_add(
                            qk_sb_cur[0][:, m, :TG], ps[:, :TG],
                            bqk_sb[:, m:m + 1]
                        )

            def emit_v(xt, g, p2s):
                v_sb, = v_sb_cur
                for p2 in p2s:
                    nb2 = min(2, g - 2 * p2)
                    if nb2 <= 0:
                        continue
                    psv = mm_ps.tile([128, 512], F32, tag="mmps")
                    for jj in range(nb2):
                        j = 2 * p2 + jj
                        for kc in range(2):
                            nc.tensor.matmul(
                                psv[:N_TOK, jj * DIM:(jj + 1) * DIM],
                                xt[:, kc, j * N_TOK:(j + 1) * N_TOK],
                                wv_sb[:, kc, :],
                                start=(kc == 0), stop=(kc == 1),
                            )
                    v_dst = v_sb[:, 2 * p2:2 * p2 + nb2, :]
                    v_src = psv[:N_TOK, :nb2 * DIM].rearrange(
                        "p (j c) -> p j c", c=DIM)
                    if p2 == 0:
                        nc.scalar.copy(v_dst, v_src)
                    else:
                        nc.vector.tensor_copy(v_dst, v_src)

            def emit_scores(qk_sb, g, f0, emult_engs):
                """One fill: batches f0..f0+nb-1, all 8 heads, into FOUR
                [128,512] tiles from the unified mm ring (4 adjacent ring
                slots = 4 distinct psum banks, as the concurrent row-band
                matmuls require: band 32*(h%4) writes tile h%4).
                slot = 2*jl + h//4.  exp+emult run per bank tile (520ns
                granularity) so urgent evacs never queue behind them."""
                nb = min(FILL, g - f0)
                vcols = 84 * 2 * nb
                s_tiles = []
                for _bank in range(4):
                    s_t = mm_ps.tile([128, 512], F32, tag="mmps")
                    s_tiles.append(s_t)
                for jl in range(nb):
                    j = f0 + jl
                    for h in range(H):
                        bank = h % 4
                        slot = 2 * jl + h // 4
                        hp = 32 * bank
                        nc.tensor.matmul(
                            s_tiles[bank][:N_TOK, 84 * slot:84 * slot + 84],
                            qk_sb[hp:hp + 32, 2 + h // 4,
                                  j * N_TOK:(j + 1) * N_TOK],
                            qk_sb[hp:hp + 32, 0 + h // 4,
                                  j * N_TOK:(j + 1) * N_TOK],
                            start=True, stop=True,
                            tile_position=(hp, 0),
                        )
                e_banks = []
                for bank in range(4):
                    e0 = e0_pool.tile([N_TOK, 504], BF16, tag="e0")
                    nc.scalar.activation(
                        e0[:, :vcols], s_tiles[bank][:N_TOK, :vcols],
                        mybir.ActivationFunctionType.Exp, scale=LOGIT_SCALE,
                    )
                    e = e_pool.tile([N_TOK, 504], BF16, tag="e")
                    emult_engs[bank].tensor_tensor(
                        e[:, :vcols], e0[:, :vcols], eb_sb[:, bank, :vcols],
                        mybir.AluOpType.mult,
                    )
                    e_banks.append(e)
                return e_banks

            def emit_av(prev, hg):
                _, g, v_sb, e_tiles = prev
                TG = g * N_TOK
                avo = mm_ps.tile([128, 512], F32, tag="mmps")
                avd = mm_ps.tile([128, 512], F32, tag="mmps")

                def eslice(j, h):
                    e = e_tiles[j // FILL][h % 4]
                    slot = 2 * (j % FILL) + h // 4
                    return e[:, 84 * slot:84 * slot + 84]

                for j in range(g):
                    for hh in range(4):
                        h = 4 * hg + hh
                        nc.tensor.matmul(
                            avo[32 * hh:32 * hh + 32, 84 * j:84 * j + 84],
                            v_sb[:, j, 32 * h:32 * h + 32],
                            eslice(j, h), start=True, stop=True,
                            tile_position=(0, 32 * hh),
                        )
                # denominators: ones-weight matmuls back to back so the
                # post-compile pass drops the redundant reloads
                for j in range(g):
                    for hh in range(4):
                        h = 4 * hg + hh
                        nc.tensor.matmul(
                            avd[32 * hh:32 * hh + 32, 84 * j:84 * j + 84],
                            ones_sb, eslice(j, h), start=True, stop=True,
                            tile_position=(0, 32 * hh),
                        )
                r_sb = ot_pool.tile([128, GROUP * N_TOK], F32, tag=f"d{hg}")
                nc.vector.reciprocal_approx_fast(r_sb[:, :TG], avd[:, :TG])
                ot = ot_pool.tile([128, GROUP * N_TOK], BF16, tag=f"ot{hg}")
                nc.vector.tensor_tensor(
                    ot[:, :TG], avo[:, :TG], r_sb[:, :TG],
                    mybir.AluOpType.mult,
                )
                return ot

            def emit_proj(prev, ot_tiles):
                g0, g, _, _ = prev
                TG = g * N_TOK
                T0 = g0 * N_TOK
                for m in range(2):
                    psy = mm_ps.tile([128, 512], F32, tag="mmps")
                    for kc in range(2):
                        nc.tensor.matmul(
                            psy[:, :TG],
                            wproj_sb[:, kc, m * 128:(m + 1) * 128],
                            ot_tiles[kc][:, :TG],
                            start=(kc == 0), stop=(kc == 1),
                        )
                    y_sb = y_pool.tile([128, GROUP * N_TOK], BF16, tag=f"y{m}")
                    if m == 0:
                        nc.scalar.copy(y_sb[:, :TG], psy[:, :TG])
                    else:
                        nc.vector.tensor_copy(y_sb[:, :TG], psy[:, :TG])
                    nc.sync.dma_start(
                        yT[m * 128:(m + 1) * 128, T0:T0 + TG],
                        y_sb[:, :TG],
                    )

            # ------- software-pipelined main loop -------
            # skew: group i runs qkv/v/scores(i), AV(i-2), proj(i-3).
            # proj(i-3) opens the group so its psum evacs land FIRST in
            # the ACT/DVE queues -- the AV psum tiles (mm ring of 4) wait
            # on them, and anywhere later they stall the PE ~1us/group.
            pipeline = []   # (g0, g, v_sb, e_tiles)
            proj_q = []     # (prev_entry, ot1, ot2)
            qk_sb_cur = [None]
            v_sb_cur = [None]

            def emit_group_front(g0, g):
                TG = g * N_TOK
                T0 = g0 * N_TOK
                if proj_q:
                    emit_proj(*proj_q.pop(0))
                xt8 = xin8.tile([128, 2, GROUP * N_TOK], QK_DT)
                nc.sync.dma_start(
                    xt8[:, :, :TG],
                    xT8[:, T0:T0 + TG].rearrange("(kc p) t -> p kc t", p=128),
                )
                xt = xinb.tile([128, 2, GROUP * N_TOK], BF16)
                nc.sync.dma_start(
                    xt[:, :, :TG],
                    xT[:, T0:T0 + TG].rearrange("(kc p) t -> p kc t", p=128),
                )
                qk_sb = qk_pool.tile([128, 4, GROUP * N_TOK], BF16)
                v_sb = v_pool.tile([N_TOK, GROUP, DIM], BF16)
                qk_sb_cur[0] = qk_sb
                v_sb_cur[0] = v_sb
                prev2 = pipeline[-2] if len(pipeline) >= 2 else None

                emit_qkv_m(xt8, g, [0, 2, 1, 3])
                ot1 = emit_av(prev2, 0) if prev2 else None
                emit_v(xt, g, [0, 1])
                e_tiles = []
                gps3 = [nc.vector, nc.gpsimd, nc.gpsimd, nc.gpsimd]
                e_tiles.append(emit_scores(qk_sb, g, 0, gps3))
                ot2 = emit_av(prev2, 1) if prev2 else None
                if prev2:
                    proj_q.append((prev2, [ot1, ot2]))
                if g > FILL:
                    e_tiles.append(emit_scores(qk_sb, g, FILL, gps3))
                emit_v(xt, g, [2])
                pipeline.append((g0, g, v_sb, e_tiles))

            for (g0, g) in groups:
                emit_group_front(g0, g)
            # drain: AV+proj for the last two groups, plus pending proj
            for prev in pipeline[-2:]:
                if proj_q:
                    emit_proj(*proj_q.pop(0))
                ot1 = emit_av(prev, 0)
                ot2 = emit_av(prev, 1)
                proj_q.append((prev, [ot1, ot2]))
            while proj_q:
                emit_proj(*proj_q.pop(0))

    nc.compile()
    if os.environ.get("K2_LDWOPT", "1") == "1":
        _optimize_ldweights(nc)
    return nc


def _sync_empty(i):
    si = i.sync_info
    return si is None or (len(si.on_wait) == 0 and len(si.on_update) == 0)


def _optimize_ldweights(nc):
    """Post-compile PE weight-load optimization: drop an Ldweights that is
    identical to what the covered array strips already hold (e.g. the
    ones-vector for the denominator matmuls)."""
    for fn in nc.m.functions:
        for blk in fn.blocks:
            insts = list(blk.instructions)
            ldw_idx = [k for k, i in enumerate(insts)
                       if i.opcode == 'Ldweights']
            drop = set()
            state = {}
            for k in ldw_idx:
                i = insts[k]
                ap = i.ins[0]
                pairs = tuple(tuple(p) for p in ap.ap)
                tp = tuple(i.tile_position) if i.tile_position else (0, 0)
                ts = tuple(i.tile_size) if i.tile_size else None
                key = (str(ap.memref), ap.offset, pairs, tp, ts)
                rows = pairs[0][1]
                cols = 1
                for p in pairs[1:]:
                    cols *= p[1]
                r0, c0 = tp
                strips = [(r, c)
                          for r in range(r0 // 32, min(4, (r0 + rows + 31) // 32))
                          for c in range(c0 // 32, min(4, (c0 + cols + 31) // 32))]
                if all(state.get(s) == key for s in strips) and _sync_empty(i):
                    drop.add(k)
                else:
                    for s in strips:
                        state[s] = key
            if drop:
                blk.instructions = [i for k, i in enumerate(insts)
                                    if k not in drop]
    return nc


def _host_prep(x, W_qkv, b_qkv, W_proj, b_proj, bias_table, rel_index,
               n_batches):
    """Build the per-core input dict pieces shared across cores."""
    bias_full = bias_table[rel_index]          # (84, 84, 8): [q, k, h]
    bias_full = np.transpose(bias_full, (2, 0, 1))  # [h, q, k]
    eb = np.zeros((N_TOK, 4, 504), dtype=np.float32)
    for bank in range(4):
        for slot in range(6):
            h = bank + 4 * (slot % 2)
            # eb[k, bank, 84*slot + q] = exp(bias[h, q, k])
            eb[:, bank, 84 * slot:84 * slot + 84] = np.exp(bias_full[h].T)
    eb = eb.astype(ml_dtypes.bfloat16)

    qk_np_dt = ml_dtypes.float8_e4m3fn if USE_FP8 else ml_dtypes.bfloat16
    shared = {
        "wqk8": np.ascontiguousarray(W_qkv[:, :2 * DIM] * WQK_SCALE
                                     ).astype(qk_np_dt),
        "wv": np.ascontiguousarray(W_qkv[:, 2 * DIM:]).astype(
            ml_dtypes.bfloat16),
        "bqk": np.ascontiguousarray(b_qkv[:2 * DIM] * WQK_SCALE,
                                    dtype=np.float32),
        "wproj": np.ascontiguousarray(W_proj).astype(ml_dtypes.bfloat16),
        "ebias": eb,
    }
    return shared


def make_in_maps(x, W_qkv, b_qkv, W_proj, b_proj, bias_table, rel_index, nb):
    shared = _host_prep(x, W_qkv, b_qkv, W_proj, b_proj, bias_table,
                        rel_index, nb)
    in_maps = []
    for c in range(N_CORES):
        xs = x[c * nb:(c + 1) * nb].reshape(nb * N_TOK, DIM)
        xTc = np.ascontiguousarray(xs.T)
        m = dict(shared)
        m["xT8"] = xTc.astype(
            ml_dtypes.float8_e4m3fn if USE_FP8 else ml_dtypes.bfloat16)
        m["xT"] = xTc.astype(ml_dtypes.bfloat16)
        in_maps.append(m)
    return in_maps


_NC_CACHE = {}


def kernel(x, W_qkv, b_qkv, W_proj, b_proj, bias_table, rel_index,
           n_batches_per_core=B_SHARD):
    x = np.asarray(x, dtype=np.float32)
    W_qkv = np.asarray(W_qkv, dtype=np.float32)
    b_qkv = np.asarray(b_qkv, dtype=np.float32)
    W_proj = np.asarray(W_proj, dtype=np.float32)
    b_proj = np.asarray(b_proj, dtype=np.float32)
    bias_table = np.asarray(bias_table, dtype=np.float32)
    rel_index = np.asarray(rel_index)

    nb = n_batches_per_core
    if nb not in _NC_CACHE:
        _NC_CACHE[nb] = build_nc(nb)
    nc = _NC_CACHE[nb]

    in_maps = make_in_maps(x, W_qkv, b_qkv, W_proj, b_proj, bias_table,
                           rel_index, nb)
    res = run_bass_kernel_spmd(nc, in_maps, core_ids=list(range(N_CORES)))

    # combined output bias: b_proj plus v-bias passed through attention
    bias_out = (b_qkv[2 * DIM:3 * DIM] @ W_proj + b_proj).astype(np.float32)

    outs = []
    for c in range(N_CORES):
        yTc = np.asarray(res.results[c]["yT"]).astype(np.float32)
        y = np.ascontiguousarray(yTc.T).reshape(nb, N_TOK, DIM)
        outs.append(y)
    out = np.concatenate(outs, axis=0)
    out += bias_out[None, None, :]
    return out.astype(np.float32)


if __name__ == "__main__":
    # smoke test with tiny batch count per core
    rng = np.random.default_rng(0)
    nb = 13
    B = N_CORES * nb
    x = rng.standard_normal((B, N_TOK, DIM), dtype=np.float32)
    W_qkv = rng.standard_normal((DIM, 3 * DIM), dtype=np.float32) * 0.02
    b_qkv = np.zeros(3 * DIM, np.float32)
    W_proj = rng.standard_normal((DIM, DIM), dtype=np.float32) * 0.02
    b_proj = np.zeros(DIM, np.float32)
    bias_table = rng.standard_normal((207, H), dtype=np.float32) * 0.02
    SQ = [64, 16, 4]
    offset = [0]
    for i in range(2):
        offset.append(sum(SQ[-i - 1:]))
    off_h = np.concatenate([np.full(SQ[i], offset[i], dtype=np.int64) for i in range(3)])
    off_w = np.concatenate([np.full(SQ[i], offset[-i - 1], dtype=np.int64) for i in range(3)])
    ch = np.arange(N_TOK)
    cw = np.arange(N_TOK)[::-1]
    rel_index = (ch[:, None] + cw[None, :] + off_h[:, None] + off_w[None, :]).astype(np.int32)

    out = kernel(x, W_qkv, b_qkv, W_proj, b_proj, bias_table, rel_index,
                 n_batches_per_core=nb)

    # numpy reference check
    qkv = (x.reshape(-1, DIM) @ W_qkv + b_qkv).reshape(B, N_TOK, 3, H, HD)
    qkv = qkv.transpose(2, 0, 3, 1, 4)
    q, k, v = qkv[0], qkv[1], qkv[2]
    attn = np.einsum('bhqd,bhkd->bhqk', q * SCALE, k)
    bias = bias_table[rel_index].transpose(2, 0, 1)
    attn = attn + bias[None]
    attn = np.exp(attn)
    attn /= attn.sum(-1, keepdims=True)
    ref = np.einsum('bhqk,bhkd->bhqd', attn, v).transpose(0, 2, 1, 3).reshape(B, N_TOK, DIM)
    ref = ref @ W_proj + b_proj
    err = np.linalg.norm(out - ref) / np.linalg.norm(ref)
    print("out", out.shape, out.dtype, "rel err", err)


# revision 3
# speedup vs baseline: 1.0328x; 1.0328x over previous
"""Trainium2 Bass kernel for Local_Scale_Attention (optimized v2).

Problem (hardcoded shapes):
  x:          (2048, 84, 256) f32
  W_qkv:      (256, 768) f32,  b_qkv: (768,) f32
  W_proj:     (256, 256) f32,  b_proj: (256,) f32
  bias_table: (207, 8) f32,    rel_index: (84, 84) i32
  out:        (2048, 84, 256) f32

Sharding: data-parallel over batch across 8 cores (256 batches/core).

Per-core pipeline (feature-major, S^T orientation), ~357us vs 539us v1:
  - q,k projections in fp8e4m3 with DoubleRow (K=256 in one matmul):
    host sends xT8 (e4m3) and W_qk*32 (e4m3); the 1024x logit scale is
    folded into the exp scale.  DR gives no col-rate gain on TRN2 but
    halves PE instructions + psum writes (~43us).  v/proj stay bf16
    (fp8 there exceeds the 2e-2 error budget; qk-fp8 only moves rel
    err 4.7e-3 -> 6.5e-3 because the logits are small).
  - ALL psum flows through one unified 8-bank mm ring (bufs=8): qkv
    m-tiles, v-tiles, S-fill bank-tiles, AV/denominator tiles, proj
    tiles.  A score fill takes 4 adjacent ring slots = 4 distinct
    banks; HW rule (probed): concurrently-streaming row-band-packed
    matmuls must write DISTINCT psum banks -> band 32*(h%4) writes
    ring tile h%4.  exp runs per bank tile (520ns chunks).
  - skew-3 software pipeline: group i emits proj(i-3) FIRST (its psum
    evacs land at the head of the engine queues), then qkv(i),
    AV(i-2, hg0), v(i), scores(i), AV(i-2, hg1).  This keeps the PE
    continuously busy so it holds the 2.4GHz p-state (idle drops it
    to 1.2GHz and inflates every matmul 2x).
  - engine assignment (ACT/DVE are the only psum-capable aux engines;
    GPSIMD is SBUF-only and ~6x slower on bulk elementwise):
    ACT = qk evac (m0/m2, bias add) + v-quad0 evac + 8 exps
    DVE = qk evac (m1/m3) + v evacs + proj evacs + reciprocal +
          O/D multiply + 2 of 8 E-mults
    GPSIMD = 6 of 8 E = E0*exp(bias) multiplies
  - denominators via ones-weight matmuls col-band-packed into the AV
    psum tiles (redundant ones Ldweights dropped post-compile).
  - y written bf16 (rel err impact ~2e-4; halves output DMA).
"""

import math
import os

import numpy as np
import ml_dtypes

USE_FP8 = os.environ.get("K2_FP8", "1") == "1"
EMULT_GPS = os.environ.get("K2_EMULT_GPS", "1") == "1"
QKEVAC_ACT = os.environ.get("K2_QKEVAC_ACT", "1") == "1"

import concourse.bass as bass
import concourse.bacc as bacc
import concourse.mybir as mybir
import concourse.tile as tile
from concourse.bass_utils import run_bass_kernel_spmd

F32 = mybir.dt.float32
BF16 = mybir.dt.bfloat16
F8 = mybir.dt.float8e4

N_CORES = 8
B_TOTAL = 2048
B_SHARD = B_TOTAL // N_CORES  # 256
N_TOK = 84
DIM = 256
H = 8
HD = 32
SCALE = float(N_TOK) ** -0.5

GROUP = 6           # batches per outer group (N = 6*84 = 504 <= 512)
FILL = 3            # batches per S-psum fill (12 S-matrices in 2 banks/half)

WQK_SCALE = 32.0 if USE_FP8 else 1.0  # fp8 pre-scale on W_q/W_k
LOGIT_SCALE = SCALE / (WQK_SCALE * WQK_SCALE)

DR = mybir.MatmulPerfMode.DoubleRow


def build_nc(n_batches=B_SHARD):
    T_ALL = n_batches * N_TOK
    nc = bacc.Bacc("TRN2", target_bir_lowering=False, debug=False)

    QK_DT = F8 if USE_FP8 else BF16
    xT8 = nc.declare_dram_parameter("xT8", [DIM, T_ALL], QK_DT, isOutput=False)
    xT = nc.declare_dram_parameter("xT", [DIM, T_ALL], BF16, isOutput=False)
    wqk8 = nc.declare_dram_parameter("wqk8", [DIM, 2 * DIM], QK_DT, isOutput=False)
    wv = nc.declare_dram_parameter("wv", [DIM, DIM], BF16, isOutput=False)
    bqk = nc.declare_dram_parameter("bqk", [2 * DIM], F32, isOutput=False)
    wproj = nc.declare_dram_parameter("wproj", [DIM, DIM], BF16, isOutput=False)
    # exp(bias) pre-aligned to the S-fill layout (bank = h%4, slot parity
    # = h//4): [84(k), 4(bank), 504(6 slots x 84 q)]
    ebias = nc.declare_dram_parameter("ebias", [N_TOK, 4, 504], BF16,
                                      isOutput=False)
    yT = nc.declare_dram_parameter("yT", [DIM, T_ALL], BF16, isOutput=True)

    groups = []
    b0 = 0
    while b0 < n_batches:
        groups.append((b0, min(GROUP, n_batches - b0)))
        b0 += GROUP

    with tile.TileContext(nc) as tc:
        with (
            tc.tile_pool(name="const", bufs=1) as const,
            tc.tile_pool(name="xin8", bufs=4) as xin8,
            tc.tile_pool(name="xinb", bufs=4) as xinb,
            tc.tile_pool(name="qk", bufs=3) as qk_pool,
            tc.tile_pool(name="vsb", bufs=4) as v_pool,
            tc.tile_pool(name="e0sb", bufs=8) as e0_pool,
            tc.tile_pool(name="esb", bufs=20) as e_pool,
            tc.tile_pool(name="otsb", bufs=3) as ot_pool,
            tc.tile_pool(name="ysb", bufs=3) as y_pool,
            tc.tile_pool(name="mm_ps", bufs=8, space="PSUM") as mm_ps,
        ):
            # ---- static tiles ----
            wqk8_sb = const.tile([128, 2, 2 * DIM], QK_DT)
            nc.sync.dma_start(wqk8_sb, wqk8.rearrange("(kc p) m -> p kc m", p=128))
            wv_sb = const.tile([128, 2, DIM], BF16)
            nc.sync.dma_start(wv_sb, wv.rearrange("(kc p) m -> p kc m", p=128))
            wproj_sb = const.tile([128, 2, DIM], BF16)
            nc.sync.dma_start(wproj_sb, wproj.rearrange("(kc p) m -> p kc m", p=128))
            bqk_sb = const.tile([128, 4], F32)
            nc.sync.dma_start(bqk_sb, bqk.rearrange("(m p) -> p m", p=128))
            eb_sb = const.tile([N_TOK, 4, 504], BF16)
            nc.sync.dma_start(eb_sb, ebias[:])
            ones_sb = const.tile([N_TOK, HD], BF16)
            nc.vector.memset(ones_sb, 1.0)

            # ---------- pipelined stage helpers ----------
            def emit_qkv_m(xt8, g, ms):
                """q/k projection bands. m=0,1: q feats 0:128/128:256;
                m=2,3: same for k.  fp8 DoubleRow does K=256 in one pass.
                Evac: even m on ACT, odd m on DVE (scores need all four)."""
                TG = g * N_TOK
                for m in ms:
                    ps = mm_ps.tile([128, 512], F32, tag="mmps")
                    if USE_FP8:
                        nc.tensor.matmul(
                            ps[:, :TG],
                            wqk8_sb[:, :, m * 128:(m + 1) * 128],
                            xt8[:, :, :TG],
                            start=True, stop=True, perf_mode=DR,
                        )
                    else:
                        for kc in range(2):
                            nc.tensor.matmul(
                                ps[:, :TG],
                                wqk8_sb[:, kc, m * 128:(m + 1) * 128],
                                xt8[:, kc, :TG],
                                start=(kc == 0), stop=(kc == 1),
                            )
                    if m % 2 == 0:
                        nc.scalar.add(
                            qk_sb_cur[0][:, m, :TG], ps[:, :TG],
                            bqk_sb[:, m:m + 1]
                        )
                    else:
                        nc.vector.tensor_scalar_add(
                            qk_sb_cur[0][:, m, :TG], ps[:, :TG],
                            bqk_sb[:, m:m + 1]
                        )

            def emit_v(xt, g, p2s):
                v_sb, = v_sb_cur
                for p2 in p2s:
                    nb2 = min(2, g - 2 * p2)
                    if nb2 <= 0:
                        continue
                    psv = mm_ps.tile([128, 512], F32, tag="mmps")
                    for jj in range(nb2):
                        j = 2 * p2 + jj
                        for kc in range(2):
                            nc.tensor.matmul(
                                psv[:N_TOK, jj * DIM:(jj + 1) * DIM],
                                xt[:, kc, j * N_TOK:(j + 1) * N_TOK],
                                wv_sb[:, kc, :],
                                start=(kc == 0), stop=(kc == 1),
                            )
                    v_dst = v_sb[:, 2 * p2:2 * p2 + nb2, :]
                    v_src = psv[:N_TOK, :nb2 * DIM].rearrange(
                        "p (j c) -> p j c", c=DIM)
                    if p2 == 0:
                        nc.scalar.copy(v_dst, v_src)
                    else:
                        nc.vector.tensor_copy(v_dst, v_src)

            def emit_scores(qk_sb, g, f0, emult_engs):
                """One fill: batches f0..f0+nb-1, all 8 heads, into FOUR
                [128,512] tiles from the unified mm ring (4 adjacent ring
                slots = 4 distinct psum banks, as the concurrent row-band
                matmuls require: band 32*(h%4) writes tile h%4).
                slot = 2*jl + h//4.  exp+emult run per bank tile (520ns
                granularity) so urgent evacs never queue behind them."""
                nb = min(FILL, g - f0)
                vcols = 84 * 2 * nb
                s_tiles = []
                for _bank in range(4):
                    s_t = mm_ps.tile([128, 512], F32, tag="mmps")
                    s_tiles.append(s_t)
                for jl in range(nb):
                    j = f0 + jl
                    for h in range(H):
                        bank = h % 4
                        slot = 2 * jl + h // 4
                        hp = 32 * bank
                        nc.tensor.matmul(
                            s_tiles[bank][:N_TOK, 84 * slot:84 * slot + 84],
                            qk_sb[hp:hp + 32, 2 + h // 4,
                                  j * N_TOK:(j + 1) * N_TOK],
                            qk_sb[hp:hp + 32, 0 + h // 4,
                                  j * N_TOK:(j + 1) * N_TOK],
                            start=True, stop=True,
                            tile_position=(hp, 0),
                        )
                e_banks = []
                for bank in range(4):
                    e0 = e0_pool.tile([N_TOK, 504], BF16, tag="e0")
                    nc.scalar.activation(
                        e0[:, :vcols], s_tiles[bank][:N_TOK, :vcols],
                        mybir.ActivationFunctionType.Exp, scale=LOGIT_SCALE,
                    )
                    e = e_pool.tile([N_TOK, 504], BF16, tag="e")
                    emult_engs[bank].tensor_tensor(
                        e[:, :vcols], e0[:, :vcols], eb_sb[:, bank, :vcols],
                        mybir.AluOpType.mult,
                    )
                    e_banks.append(e)
                return e_banks

            def emit_av(prev, hg):
                _, g, v_sb, e_tiles = prev
                TG = g * N_TOK
                avo = mm_ps.tile([128, 512], F32, tag="mmps")
                avd = mm_ps.tile([128, 512], F32, tag="mmps")

                def eslice(j, h):
                    e = e_tiles[j // FILL][h % 4]
                    slot = 2 * (j % FILL) + h // 4
                    return e[:, 84 * slot:84 * slot + 84]

                for j in range(g):
                    for hh in range(4):
                        h = 4 * hg + hh
                        nc.tensor.matmul(
                            avo[32 * hh:32 * hh + 32, 84 * j:84 * j + 84],
                            v_sb[:, j, 32 * h:32 * h + 32],
                            eslice(j, h), start=True, stop=True,
                            tile_position=(0, 32 * hh),
                        )
                # denominators: ones-weight matmuls back to back so the
                # post-compile pass drops the redundant reloads
                for j in range(g):
                    for hh in range(4):
                        h = 4 * hg + hh
                        nc.tensor.matmul(
                            avd[32 * hh:32 * hh + 32, 84 * j:84 * j + 84],
                            ones_sb, eslice(j, h), start=True, stop=True,
                            tile_position=(0, 32 * hh),
                        )
                r_sb = ot_pool.tile([128, GROUP * N_TOK], F32, tag=f"d{hg}")
                nc.vector.reciprocal_approx_fast(r_sb[:, :TG], avd[:, :TG])
                ot = ot_pool.tile([128, GROUP * N_TOK], BF16, tag=f"ot{hg}")
                nc.vector.tensor_tensor(
                    ot[:, :TG], avo[:, :TG], r_sb[:, :TG],
                    mybir.AluOpType.mult,
                )
                return ot

            def emit_proj(prev, ot_tiles):
                g0, g, _, _ = prev
                TG = g * N_TOK
                T0 = g0 * N_TOK
                for m in range(2):
                    psy = mm_ps.tile([128, 512], F32, tag="mmps")
                    for kc in range(2):
                        nc.tensor.matmul(
                            psy[:, :TG],
                            wproj_sb[:, kc, m * 128:(m + 1) * 128],
                            ot_tiles[kc][:, :TG],
                            start=(kc == 0), stop=(kc == 1),
                        )
                    y_sb = y_pool.tile([128, GROUP * N_TOK], BF16, tag=f"y{m}")
                    nc.vector.tensor_copy(y_sb[:, :TG], psy[:, :TG])
                    nc.sync.dma_start(
                        yT[m * 128:(m + 1) * 128, T0:T0 + TG],
                        y_sb[:, :TG],
                    )

            # ------- software-pipelined main loop -------
            # skew: group i runs qkv/v/scores(i), AV(i-2), proj(i-3).
            # proj(i-3) opens the group so its psum evacs land FIRST in
            # the ACT/DVE queues -- the AV psum tiles (mm ring of 4) wait
            # on them, and anywhere later they stall the PE ~1us/group.
            pipeline = []   # (g0, g, v_sb, e_tiles)
            proj_q = []     # (prev_entry, ot1, ot2)
            qk_sb_cur = [None]
            v_sb_cur = [None]

            def emit_group_front(g0, g):
                TG = g * N_TOK
                T0 = g0 * N_TOK
                if proj_q:
                    emit_proj(*proj_q.pop(0))
                xt8 = xin8.tile([128, 2, GROUP * N_TOK], QK_DT)
                nc.sync.dma_start(
                    xt8[:, :, :TG],
                    xT8[:, T0:T0 + TG].rearrange("(kc p) t -> p kc t", p=128),
                )
                xt = xinb.tile([128, 2, GROUP * N_TOK], BF16)
                nc.sync.dma_start(
                    xt[:, :, :TG],
                    xT[:, T0:T0 + TG].rearrange("(kc p) t -> p kc t", p=128),
                )
                qk_sb = qk_pool.tile([128, 4, GROUP * N_TOK], BF16)
                v_sb = v_pool.tile([N_TOK, GROUP, DIM], BF16)
                qk_sb_cur[0] = qk_sb
                v_sb_cur[0] = v_sb
                prev2 = pipeline[-2] if len(pipeline) >= 2 else None

                emit_qkv_m(xt8, g, [0, 2, 1, 3])
                ot1 = emit_av(prev2, 0) if prev2 else None
                emit_v(xt, g, [0, 1])
                e_tiles = []
                gps3 = [nc.vector, nc.gpsimd, nc.gpsimd, nc.gpsimd]
                e_tiles.append(emit_scores(qk_sb, g, 0, gps3))
                ot2 = emit_av(prev2, 1) if prev2 else None
                if prev2:
                    proj_q.append((prev2, [ot1, ot2]))
                if g > FILL:
                    e_tiles.append(emit_scores(qk_sb, g, FILL, gps3))
                emit_v(xt, g, [2])
                pipeline.append((g0, g, v_sb, e_tiles))

            for (g0, g) in groups:
                emit_group_front(g0, g)
            # drain: AV+proj for the last two groups, plus pending proj
            for prev in pipeline[-2:]:
                if proj_q:
                    emit_proj(*proj_q.pop(0))
                ot1 = emit_av(prev, 0)
                ot2 = emit_av(prev, 1)
                proj_q.append((prev, [ot1, ot2]))
            while proj_q:
                emit_proj(*proj_q.pop(0))

    nc.compile()
    if os.environ.get("K2_LDWOPT", "1") == "1":
        _optimize_ldweights(nc)
    return nc


def _sync_empty(i):
    si = i.sync_info
    return si is None or (len(si.on_wait) == 0 and len(si.on_update) == 0)


def _optimize_ldweights(nc):
    """Post-compile PE weight-load optimization: drop an Ldweights that is
    identical to what the covered array strips already hold (e.g. the
    ones-vector for the denominator matmuls)."""
    for fn in nc.m.functions:
        for blk in fn.blocks:
            insts = list(blk.instructions)
            ldw_idx = [k for k, i in enumerate(insts)
                       if i.opcode == 'Ldweights']
            drop = set()
            state = {}
            for k in ldw_idx:
                i = insts[k]
                ap = i.ins[0]
                pairs = tuple(tuple(p) for p in ap.ap)
                tp = tuple(i.tile_position) if i.tile_position else (0, 0)
                ts = tuple(i.tile_size) if i.tile_size else None
                key = (str(ap.memref), ap.offset, pairs, tp, ts)
                rows = pairs[0][1]
                cols = 1
                for p in pairs[1:]:
                    cols *= p[1]
                r0, c0 = tp
                strips = [(r, c)
                          for r in range(r0 // 32, min(4, (r0 + rows + 31) // 32))
                          for c in range(c0 // 32, min(4, (c0 + cols + 31) // 32))]
                if all(state.get(s) == key for s in strips) and _sync_empty(i):
                    drop.add(k)
                else:
                    for s in strips:
                        state[s] = key
            if drop:
                blk.instructions = [i for k, i in enumerate(insts)
                                    if k not in drop]
    return nc


def _host_prep(x, W_qkv, b_qkv, W_proj, b_proj, bias_table, rel_index,
               n_batches):
    """Build the per-core input dict pieces shared across cores."""
    bias_full = bias_table[rel_index]          # (84, 84, 8): [q, k, h]
    bias_full = np.transpose(bias_full, (2, 0, 1))  # [h, q, k]
    eb = np.zeros((N_TOK, 4, 504), dtype=np.float32)
    for bank in range(4):
        for slot in range(6):
            h = bank + 4 * (slot % 2)
            # eb[k, bank, 84*slot + q] = exp(bias[h, q, k])
            eb[:, bank, 84 * slot:84 * slot + 84] = np.exp(bias_full[h].T)
    eb = eb.astype(ml_dtypes.bfloat16)

    qk_np_dt = ml_dtypes.float8_e4m3fn if USE_FP8 else ml_dtypes.bfloat16
    shared = {
        "wqk8": np.ascontiguousarray(W_qkv[:, :2 * DIM] * WQK_SCALE
                                     ).astype(qk_np_dt),
        "wv": np.ascontiguousarray(W_qkv[:, 2 * DIM:]).astype(
            ml_dtypes.bfloat16),
        "bqk": np.ascontiguousarray(b_qkv[:2 * DIM] * WQK_SCALE,
                                    dtype=np.float32),
        "wproj": np.ascontiguousarray(W_proj).astype(ml_dtypes.bfloat16),
        "ebias": eb,
    }
    return shared


def make_in_maps(x, W_qkv, b_qkv, W_proj, b_proj, bias_table, rel_index, nb):
    shared = _host_prep(x, W_qkv, b_qkv, W_proj, b_proj, bias_table,
                        rel_index, nb)
    in_maps = []
    for c in range(N_CORES):
        xs = x[c * nb:(c + 1) * nb].reshape(nb * N_TOK, DIM)
        xTc = np.ascontiguousarray(xs.T)
        m = dict(shared)
        m["xT8"] = xTc.astype(
            ml_dtypes.float8_e4m3fn if USE_FP8 else ml_dtypes.bfloat16)
        m["xT"] = xTc.astype(ml_dtypes.bfloat16)
        in_maps.append(m)
    return in_maps


_NC_CACHE = {}


def kernel(x, W_qkv, b_qkv, W_proj, b_proj, bias_table, rel_index,
           n_batches_per_core=B_SHARD):
    x = np.asarray(x, dtype=np.float32)
    W_qkv = np.asarray(W_qkv, dtype=np.float32)
    b_qkv = np.asarray(b_qkv, dtype=np.float32)
    W_proj = np.asarray(W_proj, dtype=np.float32)
    b_proj = np.asarray(b_proj, dtype=np.float32)
    bias_table = np.asarray(bias_table, dtype=np.float32)
    rel_index = np.asarray(rel_index)

    nb = n_batches_per_core
    if nb not in _NC_CACHE:
        _NC_CACHE[nb] = build_nc(nb)
    nc = _NC_CACHE[nb]

    in_maps = make_in_maps(x, W_qkv, b_qkv, W_proj, b_proj, bias_table,
                           rel_index, nb)
    res = run_bass_kernel_spmd(nc, in_maps, core_ids=list(range(N_CORES)))

    # combined output bias: b_proj plus v-bias passed through attention
    bias_out = (b_qkv[2 * DIM:3 * DIM] @ W_proj + b_proj).astype(np.float32)

    outs = []
    for c in range(N_CORES):
        yTc = np.asarray(res.results[c]["yT"]).astype(np.float32)
        y = np.ascontiguousarray(yTc.T).reshape(nb, N_TOK, DIM)
        outs.append(y)
    out = np.concatenate(outs, axis=0)
    out += bias_out[None, None, :]
    return out.astype(np.float32)


if __name__ == "__main__":
    # smoke test with tiny batch count per core
    rng = np.random.default_rng(0)
    nb = 13
    B = N_CORES * nb
    x = rng.standard_normal((B, N_TOK, DIM), dtype=np.float32)
    W_qkv = rng.standard_normal((DIM, 3 * DIM), dtype=np.float32) * 0.02
    b_qkv = np.zeros(3 * DIM, np.float32)
    W_proj = rng.standard_normal((DIM, DIM), dtype=np.float32) * 0.02
    b_proj = np.zeros(DIM, np.float32)
    bias_table = rng.standard_normal((207, H), dtype=np.float32) * 0.02
    SQ = [64, 16, 4]
    offset = [0]
    for i in range(2):
        offset.append(sum(SQ[-i - 1:]))
    off_h = np.concatenate([np.full(SQ[i], offset[i], dtype=np.int64) for i in range(3)])
    off_w = np.concatenate([np.full(SQ[i], offset[-i - 1], dtype=np.int64) for i in range(3)])
    ch = np.arange(N_TOK)
    cw = np.arange(N_TOK)[::-1]
    rel_index = (ch[:, None] + cw[None, :] + off_h[:, None] + off_w[None, :]).astype(np.int32)

    out = kernel(x, W_qkv, b_qkv, W_proj, b_proj, bias_table, rel_index,
                 n_batches_per_core=nb)

    # numpy reference check
    qkv = (x.reshape(-1, DIM) @ W_qkv + b_qkv).reshape(B, N_TOK, 3, H, HD)
    qkv = qkv.transpose(2, 0, 3, 1, 4)
    q, k, v = qkv[0], qkv[1], qkv[2]
    attn = np.einsum('bhqd,bhkd->bhqk', q * SCALE, k)
    bias = bias_table[rel_index].transpose(2, 0, 1)
    attn = attn + bias[None]
    attn = np.exp(attn)
    attn /= attn.sum(-1, keepdims=True)
    ref = np.einsum('bhqk,bhkd->bhqd', attn, v).transpose(0, 2, 1, 3).reshape(B, N_TOK, DIM)
    ref = ref @ W_proj + b_proj
    err = np.linalg.norm(out - ref) / np.linalg.norm(ref)
    print("out", out.shape, out.dtype, "rel err", err)
